# revision 17
# baseline (speedup 1.0000x reference)
"""Distributed Bass kernel for nn_Attention_12953621365048 (8 TRN2 NeuronCores).

Sharding: 2 batch-groups x 4 head-groups (3 heads/core).
  core c: batch b = c//4, heads 3*(c%4) .. 3*(c%4)+2
Per core: qkv/kv matmuls (transposed [dim, tok] layout), RMSNorm + RoPE,
attention with no-max softmax (scores bounded: q,k RMSNorm'd), then one
8-way AllToAll per head (wrong-batch duplicate blocks zeroed via per-core
m0/m1 sender masks) to turn head-sharding into token-sharding; receiver
folds the two batch halves (gpsimd) and runs a 12-tile projection against
head-permuted Wproj.
y-token bias folding: instead of adding log(w) to scores, v rows and the
softmax-denominator tree leaves are scaled by w (identical math, bias-free
1024-wide exps on the scalar engine).
Softmax denominator: single den matmul per (head,chunk) off a DVE add-tree,
broadcast via a K=1 matmul + fast reciprocal.
Queue discipline: o1/o2 A2A-feed DMAs on sync; a2a triggers, pj gather DMAs
(which wait on collectives) and the batch-half folds all on gpsimd so a slow
A2A can't stall attention's vector/sync work.
Each per-head A2A fires as soon as its head's outputs are written; all proj
matmuls run after attention, hiding the last collective under ~38us of PE
work. Proj bias is folded into the accumulator chain (init acc = pps + bias,
final add writes bf16 directly) to kill the serialized vector tail.
Host side only shards/gathers (transpose/concat/slice).
"""

from contextlib import ExitStack

import numpy as np
import ml_dtypes

import concourse.bass as bass
import concourse.mybir as mybir
import concourse.tile as tile
from concourse import bacc
from concourse.bass_utils import run_bass_kernel_spmd

B, N, M, C, H, HD, RD = 2, 2048, 512, 1536, 12, 128, 64
EPS = 1e-6
NHL = 3               # heads per core
S = N + M             # 2560 kv tokens
KT = S // 128         # 20 kv tiles
NQC = N // 512        # 4 q-chunks of 512 (== A2A block count)
CH = 1024             # qkv-phase token chunk (bf16 moving limit)
F32 = mybir.dt.float32
F32R = mybir.dt.float32r
AF = mybir.ActivationFunctionType
ALU = mybir.AluOpType
BF16 = mybir.dt.bfloat16
NCT = C // 128        # 12 contraction tiles


def build_nc(variant=None):
    variant = variant or {}
    startup_split = variant.get("startup_split", True)
    wleaf = variant.get("wleaf", True)
    nc = bacc.Bacc("TRN2", target_bir_lowering=False, debug=False, num_devices=8)

    xT = nc.dram_tensor("xT", [C, N], BF16, kind="ExternalInput").ap()
    yT = nc.dram_tensor("yT", [C, M], BF16, kind="ExternalInput").ap()
    wqkv = nc.dram_tensor("wqkv", [C, 3 * NHL * HD], BF16, kind="ExternalInput").ap()
    wkv = nc.dram_tensor("wkv", [C, 2 * NHL * HD], BF16, kind="ExternalInput").ap()
    wproj = nc.dram_tensor("wproj", [C, C], BF16, kind="ExternalInput").ap()
    wq = nc.dram_tensor("wq", [1, HD], F32, kind="ExternalInput").ap()
    wk = nc.dram_tensor("wk", [1, HD], F32, kind="ExternalInput").ap()
    cs = nc.dram_tensor("cs", [RD, N], BF16, kind="ExternalInput").ap()
    sn = nc.dram_tensor("sn", [RD, N], BF16, kind="ExternalInput").ap()
    ywT = nc.dram_tensor("ywT", [128, M // 128], F32, kind="ExternalInput").ap()
    bpr = nc.dram_tensor("bpr", [1, C], F32, kind="ExternalInput").ap()
    onesb = nc.dram_tensor("onesb", [128, 1], BF16, kind="ExternalInput").ap()
    ones128 = nc.dram_tensor("ones128", [1, 128], F32R, kind="ExternalInput").ap()
    lywd = nc.dram_tensor("lywd", [128, M // 128], F32, kind="ExternalInput").ap()
    m0d = nc.dram_tensor("m0d", [128, 1], F32, kind="ExternalInput").ap()
    m1d = nc.dram_tensor("m1d", [128, 1], F32, kind="ExternalInput").ap()
    out = nc.dram_tensor("out", [512, C], BF16, kind="ExternalOutput").ap()

    with tile.TileContext(nc) as tc, ExitStack() as ctx:
        # ---------- outer (whole-kernel) pools ----------
        pers = ctx.enter_context(tc.tile_pool(name="persist", bufs=1))
        dram = ctx.enter_context(tc.tile_pool(name="dram", bufs=1, space="DRAM"))

        onesb_sb = pers.tile([128, 1], BF16, tag="onesb")
        nc.gpsimd.dma_start(onesb_sb[:], onesb)
        ones128_sb = pers.tile([1, 128], F32R, tag="ones128")
        nc.gpsimd.dma_start(ones128_sb[:], ones128)
        eps_sb = pers.tile([1, 1], F32, tag="eps")
        nc.vector.memset(eps_sb[:], EPS)
        wq_sb = pers.tile([128, 1], F32, tag="wq")
        nc.gpsimd.dma_start(wq_sb[:], wq.rearrange("o p -> p o"))
        wk_sb = pers.tile([128, 1], F32, tag="wk")
        nc.gpsimd.dma_start(wk_sb[:], wk.rearrange("o p -> p o"))
        m0_sb = pers.tile([128, 1], F32, tag="m0")
        nc.gpsimd.dma_start(m0_sb[:], m0d)
        m1_sb = pers.tile([128, 1], F32, tag="m1")
        nc.gpsimd.dma_start(m1_sb[:], m1d)

        # y token weights, one column per y kv tile; clamped on host
        ywT_sb = pers.tile([128, M // 128], F32, tag="ywT")
        nc.gpsimd.dma_start(ywT_sb[:], ywT)
        if not wleaf:
            # bias columns per y kv tile: log(clip(w)) computed on host
            lyw_sb = pers.tile([128, M // 128], F32, tag="lyw")
            nc.gpsimd.dma_start(lyw_sb[:], lywd)

        # persistent activations
        qn = [pers.tile([128, N], BF16, tag=f"qn{t}", name=f"qn{t}") for t in range(NHL)]
        kn = [pers.tile([128, S], BF16, tag=f"kn{t}", name=f"kn{t}") for t in range(NHL)]
        v_sb = pers.tile([128, KT * NHL * HD], BF16, tag="v")  # [kv_tile, head, hd]

        outp = ctx.enter_context(tc.tile_pool(name="osb", bufs=3))

        # ---------- phase A/B: qkv + kv, norm, rope ----------
        with ExitStack() as ab:
            csn = ab.enter_context(tc.tile_pool(name="csn", bufs=1))
            wbig = ab.enter_context(tc.tile_pool(name="wbig", bufs=1))
            xtp = ab.enter_context(tc.tile_pool(name="xt", bufs=2))
            sqp = ab.enter_context(tc.tile_pool(name="sq", bufs=2))
            smallp = ab.enter_context(tc.tile_pool(name="small", bufs=3))
            brp = ab.enter_context(tc.tile_pool(name="bcast", bufs=2))
            ropep = ab.enter_context(tc.tile_pool(name="rope", bufs=2))
            psA = ab.enter_context(tc.tile_pool(name="psA", bufs=2, space="PSUM"))
            psV = ab.enter_context(tc.tile_pool(name="psV", bufs=2, space="PSUM"))
            psS = ab.enter_context(tc.tile_pool(name="psS", bufs=1, space="PSUM"))

            def norm_head(raw_ps, dst, w_sb, rope_q0, CHc):
                """RMSNorm over partition dim (HD) + optional RoPE; [128,CHc]."""
                sq = sqp.tile([128, CH], BF16, tag="sq", name="sq")[:, :CHc]
                nc.scalar.activation(sq, raw_ps[:], AF.Square)
                ssq = psS.tile([1, CH], F32, tag="ssq", name="ssq")[:, :CHc]
                for h0 in range(0, CHc, 512):
                    hw = min(512, CHc - h0)
                    nc.tensor.matmul(
                        ssq[:, h0 : h0 + hw],
                        onesb_sb[:],
                        sq[:, h0 : h0 + hw],
                        start=True,
                        stop=True,
                    )
                inv = smallp.tile([1, CH], F32, tag="inv", name="inv")[:, :CHc]
                nc.scalar.activation(
                    inv, ssq, AF.Abs_reciprocal_sqrt, bias=eps_sb[:],
                    scale=1.0 / HD,
                )
                binv = brp.tile([128, CH], F32, tag="binv", name="binv")[:, :CHc]
                nc.gpsimd.partition_broadcast(binv, inv)
                nc.vector.scalar_tensor_tensor(
                    dst, raw_ps[:], w_sb[:], binv, op0=ALU.mult, op1=ALU.mult
                )
                if rope_q0 is not None:
                    hf = RD // 2
                    csc = cs_sb[:, rope_q0 : rope_q0 + CHc]
                    snc = sn_sb[:, rope_q0 : rope_q0 + CHc]
                    sw = ropep.tile([RD, CH], BF16, tag="sw", name="sw")[:, :CHc]
                    nc.scalar.copy(sw[0:hf, :], dst[hf:RD, :])
                    nc.scalar.copy(sw[hf:RD, :], dst[0:hf, :])
                    ma = ropep.tile([RD, CH], BF16, tag="ma", name="ma")[:, :CHc]
                    mb = ropep.tile([RD, CH], BF16, tag="mb", name="mb")[:, :CHc]
                    nc.vector.tensor_mul(ma, dst[0:RD, :], csc)
                    nc.vector.tensor_mul(mb, sw, snc)
                    nc.vector.tensor_add(dst[0:RD, :], ma, mb)

            def v_chunk(src_sb, w_sb, nqh, vt0, CHc, vscale):
                """v heads for one chunk; vscale: per-kv-tile weight cols or None."""
                voff = (nqh + NHL) * HD
                for ts in range(CHc // 128):
                    ps = psV.tile([128, NHL * HD], F32, tag="vps")
                    for ct in range(NCT):
                        nc.tensor.matmul(
                            ps[:],
                            src_sb[:, ct, ts * 128 : (ts + 1) * 128],
                            w_sb[:, ct, voff : voff + NHL * HD],
                            start=(ct == 0),
                            stop=(ct == NCT - 1),
                        )
                    kvt = vt0 + ts
                    dst = v_sb[:, kvt * NHL * HD : (kvt + 1) * NHL * HD]
                    if vscale is not None:
                        nc.vector.tensor_scalar_mul(dst, ps[:], vscale[:, ts : ts + 1])
                    else:
                        nc.vector.tensor_copy(dst, ps[:])

            def qkv_chunk(src_sb, w_sb, nqh, q0, kdst_off, vt0, rope, CHc,
                          vscale=None, vfirst=False):
                """One CHc-token chunk: q (nqh heads), k (NHL heads), v (NHL heads)."""
                if vfirst:
                    v_chunk(src_sb, w_sb, nqh, vt0, CHc, vscale)
                for t in range(nqh):
                    ps = psA.tile([128, CH], F32, tag="qk", name="qk")[:, :CHc]
                    for ct in range(NCT):
                        for h0 in range(0, CHc, 512):
                            hw = min(512, CHc - h0)
                            nc.tensor.matmul(
                                ps[:, h0 : h0 + hw],
                                w_sb[:, ct, t * HD : (t + 1) * HD],
                                src_sb[:, ct, h0 : h0 + hw],
                                start=(ct == 0),
                                stop=(ct == NCT - 1),
                            )
                    norm_head(
                        ps, qn[t][:, q0 : q0 + CHc], wq_sb,
                        q0 if rope else None, CHc,
                    )
                koff = nqh * HD
                for t in range(NHL):
                    ps = psA.tile([128, CH], F32, tag="qk", name="qk")[:, :CHc]
                    for ct in range(NCT):
                        for h0 in range(0, CHc, 512):
                            hw = min(512, CHc - h0)
                            nc.tensor.matmul(
                                ps[:, h0 : h0 + hw],
                                w_sb[:, ct, koff + t * HD : koff + (t + 1) * HD],
                                src_sb[:, ct, h0 : h0 + hw],
                                start=(ct == 0),
                                stop=(ct == NCT - 1),
                            )
                    norm_head(
                        ps,
                        kn[t][:, kdst_off : kdst_off + CHc],
                        wk_sb,
                        q0 if rope else None,
                        CHc,
                    )
                if not vfirst:
                    v_chunk(src_sb, w_sb, nqh, vt0, CHc, vscale)

            xt_first = xtp.tile([128, NCT, CH], BF16, tag="xt", name="xt_first")
            wqkv_sb = wbig.tile([128, NCT, 3 * NHL * HD], BF16, tag="wbig")
            # wkv has its own buffer and loads early (vector queue) so the
            # y phase isn't gated on a late weight fetch
            wkv_sb = wbig.tile([128, NCT, 2 * NHL * HD], BF16, tag="wkv")
            if startup_split:
                # tiny first transfers so the first matmul starts ASAP;
                # wqkv slices alternate sync/vector so two DMA queues feed
                # the startup-critical weights in parallel
                nc.sync.dma_start(wqkv_sb[:, 0, 0:HD], wqkv[0:128, 0:HD])
                nc.scalar.dma_start(xt_first[:, 0, 0:512], xT[0:128, 0:512])
                nc.sync.dma_start(
                    wqkv_sb[:, 0, HD : 3 * NHL * HD],
                    wqkv[0:128, HD : 3 * NHL * HD],
                )
                nc.scalar.dma_start(xt_first[:, 0, 512:CH], xT[0:128, 512:CH])
                for ct in range(1, NCT):
                    nc.sync.dma_start(
                        wqkv_sb[:, ct, :], wqkv[ct * 128 : (ct + 1) * 128, :]
                    )
                    nc.scalar.dma_start(
                        xt_first[:, ct, :],
                        xT[ct * 128 : (ct + 1) * 128, 0:CH],
                    )
            else:
                for ct in range(NCT):
                    nc.sync.dma_start(
                        wqkv_sb[:, ct, :], wqkv[ct * 128 : (ct + 1) * 128, :]
                    )
                    nc.scalar.dma_start(
                        xt_first[:, ct, :],
                        xT[ct * 128 : (ct + 1) * 128, 0:CH],
                    )
            cs_sb = csn.tile([RD, N], BF16, tag="cs")
            nc.gpsimd.dma_start(cs_sb[:], cs)
            sn_sb = csn.tile([RD, N], BF16, tag="sn")
            nc.gpsimd.dma_start(sn_sb[:], sn)
            # single rearranged transfer: cheap to issue, lands well before
            # the y phase needs it
            nc.gpsimd.dma_start(
                wkv_sb[:], wkv.rearrange("(ct p) q -> p ct q", p=128)
            )
            for qc in range(N // CH):
                q0 = qc * CH
                if qc == 0:
                    xt_sb = xt_first
                else:
                    xt_sb = xtp.tile([128, NCT, CH], BF16, tag="xt", bufs=2)
                    nc.sync.dma_start(
                        xt_sb[:],
                        xT[:, q0 : q0 + CH].rearrange("(ct p) q -> p ct q", p=128),
                    )
                qkv_chunk(xt_sb, wqkv_sb, NHL, q0, q0, q0 // 128, rope=True, CHc=CH)
            yt_sb = xtp.tile([128, NCT, CH], BF16, tag="xt")
            nc.sync.dma_start(
                yt_sb[:, :, :M], yT.rearrange("(ct p) q -> p ct q", p=128)
            )
            qkv_chunk(
                yt_sb, wkv_sb, 0, 0, N, N // 128, rope=False, CHc=M,
                vscale=(ywT_sb if wleaf else None), vfirst=True,
            )

        # ---------- phase C: attention + per-head A2A, then projection ----------
        with ExitStack() as pc:
            expp = pc.enter_context(tc.tile_pool(name="exp", bufs=6))
            exsp = pc.enter_context(tc.tile_pool(name="exs", bufs=3))
            brp2 = pc.enter_context(tc.tile_pool(name="bcast2", bufs=2))
            smallc = pc.enter_context(tc.tile_pool(name="smallc", bufs=2))
            accp = pc.enter_context(tc.tile_pool(name="accp", bufs=1))
            pjp = pc.enter_context(tc.tile_pool(name="pjp", bufs=1))
            wpre = pc.enter_context(tc.tile_pool(name="wpre", bufs=2))

            bpr_sb = pjp.tile([1, C], F32, tag="bpr")
            nc.sync.dma_start(bpr_sb[:], bpr)
            bb_sb = pjp.tile([128, C], F32, tag="bb")
            nc.gpsimd.partition_broadcast(bb_sb[:], bpr_sb[:])

            a2a_ins = [
                dram.tile([2 * NQC, 128, 512], BF16, name=f"a2ai{t}") for t in range(NHL)
            ]
            a2a_outs = [
                dram.tile([2 * NQC, 128, 512], BF16, name=f"a2ao{t}") for t in range(NHL)
            ]
            acc = [
                accp.tile([128, 512], F32, tag=f"acc{i}", name=f"acc{i}")
                for i in range(12)
            ]
            pj = [None] * NHL

            def prefetch_w(t):
                wp = wpre.tile(
                    [128, 12, 512], BF16, tag="wpre", bufs=3, name=f"wpre{t}",
                )
                for i in range(NQC):
                    nc.sync.dma_start(
                        wp[:, 3 * i : 3 * (i + 1), :],
                        wproj[t * 512 + i * 128 : t * 512 + (i + 1) * 128, :],
                    )
                return wp

            with ExitStack() as aps:
                psSc = aps.enter_context(tc.tile_pool(name="psSc", bufs=2, space="PSUM"))
                psAv = aps.enter_context(tc.tile_pool(name="psAv", bufs=2, space="PSUM"))
                psDen = aps.enter_context(tc.tile_pool(name="psDen", bufs=1, space="PSUM"))

                def attention_head(t):
                    for qc in range(NQC):
                        av = psAv.tile([128, 512], F32, tag="av")
                        den = psDen.tile([1, 512], F32, tag="den")
                        pair_exs = []
                        quad_exs = []
                        for kp in range(KT // 2):
                            sc = psSc.tile([128, 1024], F32, tag="sc")
                            for kh in range(2):
                                kt = 2 * kp + kh
                                nc.tensor.matmul(
                                    sc[:, kh * 512 : (kh + 1) * 512],
                                    kn[t][:, kt * 128 : (kt + 1) * 128],
                                    qn[t][:, qc * 512 : (qc + 1) * 512],
                                    start=True,
                                    stop=True,
                                )
                            ex = expp.tile([128, 1024], BF16, tag="ex", bufs=9)
                            if wleaf or kp < 8:
                                nc.scalar.activation(ex[:], sc[:], AF.Exp)
                            else:
                                for kh in range(2):
                                    kt = 2 * kp + kh
                                    nc.scalar.activation(
                                        ex[:, kh * 512 : (kh + 1) * 512],
                                        sc[:, kh * 512 : (kh + 1) * 512],
                                        AF.Exp,
                                        bias=lyw_sb[:, kt - 16 : kt - 15],
                                    )
                            for kh in range(2):
                                kt = 2 * kp + kh
                                nc.tensor.matmul(
                                    av[:],
                                    v_sb[
                                        :,
                                        kt * NHL * HD
                                        + t * HD : kt * NHL * HD
                                        + (t + 1) * HD,
                                    ],
                                    ex[:, kh * 512 : (kh + 1) * 512],
                                    start=(kt == 0),
                                    stop=(kt == KT - 1),
                                )
                            exs = exsp.tile([128, 512], BF16, tag="exs", bufs=4)
                            if wleaf and kp >= 8:
                                # w-weighted leaf: exs = ex_a*w_a + ex_b*w_b
                                ca = 2 * (kp - 8)
                                tmp = exsp.tile([128, 512], BF16, tag="ytmp", bufs=2)
                                nc.vector.tensor_scalar_mul(
                                    tmp[:], ex[:, 512:1024],
                                    ywT_sb[:, ca + 1 : ca + 2],
                                )
                                nc.vector.scalar_tensor_tensor(
                                    exs[:], ex[:, 0:512],
                                    ywT_sb[:, ca : ca + 1], tmp[:],
                                    op0=ALU.mult, op1=ALU.add,
                                )
                            else:
                                nc.vector.tensor_add(
                                    exs[:], ex[:, 0:512], ex[:, 512:1024]
                                )
                            pair_exs.append(exs)
                            if len(pair_exs) == 2:
                                exq = exsp.tile([128, 512], BF16, tag="exq", bufs=5)
                                nc.vector.tensor_add(
                                    exq[:], pair_exs[0][:], pair_exs[1][:]
                                )
                                pair_exs.clear()
                                quad_exs.append(exq)
                        # reduce the 5 quads on DVE, then a single den matmul
                        while len(quad_exs) > 1:
                            nxt = []
                            for a, b in zip(quad_exs[0::2], quad_exs[1::2]):
                                s = exsp.tile([128, 512], BF16, tag="exo", bufs=3)
                                nc.vector.tensor_add(s[:], a[:], b[:])
                                nxt.append(s)
                            if len(quad_exs) % 2:
                                nxt.append(quad_exs[-1])
                            quad_exs = nxt
                        nc.tensor.matmul(
                            den[:], onesb_sb[:], quad_exs[0][:], start=True, stop=True
                        )
                        den_sb = smallc.tile([1, 512], F32R, tag="den_sb", bufs=2)
                        nc.vector.tensor_copy(den_sb[:], den[:])
                        # broadcast den across partitions via K=1 matmul
                        bden = psDen.tile([128, 512], F32, tag="den")
                        nc.tensor.matmul(
                            bden[:], ones128_sb[:], den_sb[:], start=True, stop=True
                        )
                        binv = brp2.tile([128, 512], F32, tag="binv")
                        nc.vector.reciprocal_approx_fast(binv[:], bden[:])
                        # o1/o2: per-core batch masks m0/m1 zero the wrong-batch copy
                        o1 = outp.tile([128, 512], BF16, tag="o", bufs=4)
                        nc.vector.scalar_tensor_tensor(
                            o1[:], av[:], m0_sb[:], binv[:],
                            op0=ALU.mult, op1=ALU.mult,
                        )
                        o2 = outp.tile([128, 512], BF16, tag="o", bufs=4)
                        nc.vector.scalar_tensor_tensor(
                            o2[:], av[:], m1_sb[:], binv[:],
                            op0=ALU.mult, op1=ALU.mult,
                        )
                        nc.sync.dma_start(a2a_ins[t][qc], o1[:])
                        nc.sync.dma_start(a2a_ins[t][NQC + qc], o2[:])

                folds = [None] * NHL

                def a2a_head(t):
                    nc.gpsimd.collective_compute(
                        "AllToAll",
                        ALU.bypass,
                        replica_groups=[[0, 1, 2, 3, 4, 5, 6, 7]],
                        ins=[a2a_ins[t].opt()],
                        outs=[a2a_outs[t].opt()],
                    )
                    # gather + batch-half fold: the wait-for-collective goes
                    # on queues that can't stall attention's vector/sync work.
                    # Halves (blocks i,i+4 pairs) so the fold can start as
                    # soon as its half of the gather lands.
                    pj_t = pjp.tile(
                        [128, 2 * NQC, 512], BF16, tag="pj", bufs=2, name=f"pj{t}"
                    )
                    hw = NQC // 2
                    last = t == NHL - 1
                    # for the last head the two gather halves ride separate
                    # queues (scalar is idle post-attention) to halve latency
                    for h0 in (0, hw):
                        eng = nc.scalar if (last and h0 == 0) else nc.gpsimd
                        eng.dma_start(
                            pj_t[:, h0 : h0 + hw, :],
                            a2a_outs[t][h0 : h0 + hw].rearrange("i p q -> p i q"),
                        )
                        eng.dma_start(
                            pj_t[:, NQC + h0 : NQC + h0 + hw, :],
                            a2a_outs[t][NQC + h0 : NQC + h0 + hw].rearrange(
                                "i p q -> p i q"
                            ),
                        )
                    pjf_t = pjp.tile(
                        [128, NQC, 512], BF16, tag=f"pjf{t}", name=f"pjf{t}"
                    )

                    def fold(eng):
                        for h0 in (0, hw):
                            eng.tensor_add(
                                pjf_t[:, h0 : h0 + hw, :],
                                pj_t[:, h0 : h0 + hw, :],
                                pj_t[:, NQC + h0 : NQC + h0 + hw, :],
                            )

                    if last:
                        # deferred: emitted on vector after proj1's acc adds
                        folds[t] = fold
                    else:
                        fold(nc.gpsimd)
                    pj[t] = pjf_t

                wp0 = prefetch_w(0)
                attention_head(0)
                a2a_head(0)
                wp1 = prefetch_w(1)
                attention_head(1)
                a2a_head(1)
                wp2 = prefetch_w(2)
                attention_head(2)
                a2a_head(2)

            wps = [wp0, wp1, wp2]
            with ExitStack() as pps_ctx:
                psP = pps_ctx.enter_context(
                    tc.tile_pool(name="psP", bufs=2, space="PSUM")
                )

                def proj_partial(t):
                    wp = wps[t]
                    pjf = pj[t]
                    for fc in range(3):
                        for th in range(2):
                            pps = [
                                psP.tile(
                                    [128, 512], F32, tag=f"pp{tp_}",
                                    name=f"pp{t}_{fc}_{th}_{tp_}",
                                )
                                for tp_ in range(2)
                            ]
                            for i in range(NQC):
                                for tp_ in range(2):
                                    tcc = th * 2 + tp_
                                    nc.tensor.matmul(
                                        pps[tp_][:],
                                        pjf[:, i, tcc * 128 : (tcc + 1) * 128],
                                        wp[:, 3 * i + fc, :],
                                        start=(i == 0),
                                        stop=(i == NQC - 1),
                                    )
                            for tp_ in range(2):
                                tcc = th * 2 + tp_
                                a = acc[fc * 4 + tcc]
                                if t == 0:
                                    # fold proj bias into the init add
                                    nc.vector.tensor_tensor(
                                        a[:], pps[tp_][:],
                                        bb_sb[:, fc * 512 : (fc + 1) * 512],
                                        ALU.add,
                                    )
                                elif t == 1:
                                    nc.vector.tensor_add(a[:], a[:], pps[tp_][:])
                                else:
                                    ob = outp.tile([128, 512], BF16, tag="ob")
                                    nc.vector.tensor_add(ob[:], a[:], pps[tp_][:])
                                    nc.sync.dma_start(
                                        out[
                                            tcc * 128 : (tcc + 1) * 128,
                                            fc * 512 : (fc + 1) * 512,
                                        ],
                                        ob[:],
                                    )

                proj_partial(0)
                proj_partial(1)
                # last head's batch-half fold on the now-idle vector queue,
                # after proj0/proj1's acc adds so it can't stall them
                folds[2](nc.vector)
                proj_partial(2)
    nc.compile()
    return nc


_NC_CACHE = {}


def _get_nc(variant=None):
    key = str(sorted((variant or {}).items()))
    if key not in _NC_CACHE:
        _NC_CACHE[key] = build_nc(variant)
    return _NC_CACHE[key]


def make_in_maps(x, y, pos, y_token_weights, Wqkv, Wkv, q_norm_w, k_norm_w, Wproj, bproj):
    f = np.float32
    c32 = pos[:, :, 0].T
    s32 = pos[:, :, 1].T
    csT = np.ascontiguousarray(
        np.concatenate([c32, c32], 0).astype(ml_dtypes.bfloat16))   # [64, N]
    snT = np.ascontiguousarray(
        np.concatenate([-s32, s32], 0).astype(ml_dtypes.bfloat16))  # [64, N]
    wqs = (np.asarray(q_norm_w, dtype=f) * np.float32(HD) ** -0.5).reshape(1, HD)
    wkk = np.asarray(k_norm_w, dtype=f).reshape(1, HD)
    Wp = np.asarray(Wproj, dtype=f)
    # head-permuted Wproj: row block (t, j) = rows of head 3*j+t (same all cores)
    W = np.zeros((NHL, NQC, 128, C), dtype=f)
    for t in range(NHL):
        for j in range(NQC):
            h = 3 * j + t
            W[t, j] = Wp[h * 128 : (h + 1) * 128, :]
    wproj_perm = np.ascontiguousarray(
        W.reshape(NHL * NQC * 128, C).astype(ml_dtypes.bfloat16)
    )
    in_maps = []
    for c in range(8):
        b, g = c // 4, c % 4
        heads = [3 * g + i for i in range(NHL)]
        qcols = [Wqkv[:, h * HD : (h + 1) * HD] for h in heads]
        kcols = [Wqkv[:, C + h * HD : C + (h + 1) * HD] for h in heads]
        vcols = [Wqkv[:, 2 * C + h * HD : 2 * C + (h + 1) * HD] for h in heads]
        wqkv_c = np.ascontiguousarray(
            np.concatenate(qcols + kcols + vcols, axis=1), dtype=f
        )
        kcols2 = [Wkv[:, h * HD : (h + 1) * HD] for h in heads]
        vcols2 = [Wkv[:, C + h * HD : C + (h + 1) * HD] for h in heads]
        wkv_c = np.ascontiguousarray(np.concatenate(kcols2 + vcols2, axis=1), dtype=f)
        yw = np.clip(np.asarray(y_token_weights, dtype=f)[b], 1e-4, None)
        ywc = np.ascontiguousarray(yw.reshape(M // 128, 128).T, dtype=f)
        in_maps.append(
            {
                "xT": np.ascontiguousarray(np.asarray(x)[b].T.astype(ml_dtypes.bfloat16)),
                "yT": np.ascontiguousarray(np.asarray(y)[b].T.astype(ml_dtypes.bfloat16)),
                "wqkv": wqkv_c.astype(ml_dtypes.bfloat16),
                "wkv": wkv_c.astype(ml_dtypes.bfloat16),
                "wproj": wproj_perm,
                "wq": np.ascontiguousarray(wqs),
                "wk": np.ascontiguousarray(wkk),
                "cs": csT,
                "sn": snT,
                "ywT": ywc,
                "lywd": np.ascontiguousarray(np.log(ywc)),
                "bpr": np.asarray(bproj, dtype=f).reshape(1, C),
                "onesb": np.ones((128, 1), dtype=ml_dtypes.bfloat16),
                "ones128": np.ones((1, 128), dtype=f),
                "m0d": np.full((128, 1), 1.0 if b == 0 else 0.0, dtype=f),
                "m1d": np.full((128, 1), 0.0 if b == 0 else 1.0, dtype=f),
            }
        )
    return in_maps


def kernel(x, y, pos, y_token_weights, Wqkv, Wkv, q_norm_w, k_norm_w, Wproj, bproj,
           _trace=False, _variant=None):
    x = np.asarray(x, dtype=np.float32)
    y = np.asarray(y, dtype=np.float32)
    pos = np.asarray(pos, dtype=np.float32)
    y_token_weights = np.asarray(y_token_weights, dtype=np.float32)
    nc = _get_nc(_variant)
    in_maps = make_in_maps(
        x, y, pos, y_token_weights,
        np.asarray(Wqkv), np.asarray(Wkv), np.asarray(q_norm_w),
        np.asarray(k_norm_w), np.asarray(Wproj), np.asarray(bproj),
    )
    res = run_bass_kernel_spmd(nc, in_maps, core_ids=list(range(8)), trace=_trace)
    outp = np.zeros((B, N, C), dtype=np.float32)
    for c in range(8):
        b, g = c // 4, c % 4
        outp[b, g * 512 : (g + 1) * 512, :] = np.asarray(
            res.results[c]["out"], dtype=np.float32
        )
    if _trace:
        return outp, res
    return outp


# revision 20
# speedup vs baseline: 1.0056x; 1.0056x over previous
"""Distributed Bass kernel for nn_Attention_12953621365048 (8 TRN2 NeuronCores).

Sharding: 2 batch-groups x 4 head-groups (3 heads/core).
  core c: batch b = c//4, heads 3*(c%4) .. 3*(c%4)+2
Per core: qkv/kv matmuls (transposed [dim, tok] layout), RMSNorm + RoPE,
attention with no-max softmax (scores bounded: q,k RMSNorm'd), then one
8-way AllToAll per head (wrong-batch duplicate blocks zeroed via per-core
m0/m1 sender masks) to turn head-sharding into token-sharding; receiver
folds the two batch halves (gpsimd) and runs a 12-tile projection against
head-permuted Wproj.
y-token bias folding: instead of adding log(w) to scores, v rows and the
softmax-denominator tree leaves are scaled by w (identical math, bias-free
1024-wide exps on the scalar engine).
Softmax denominator: single den matmul per (head,chunk) off a DVE add-tree,
broadcast via a K=1 matmul + fast reciprocal.
Queue discipline: o1/o2 A2A-feed DMAs on sync; a2a triggers, pj gather DMAs
(which wait on collectives) and the batch-half folds all on gpsimd so a slow
A2A can't stall attention's vector/sync work.
Each per-head A2A fires as soon as its head's outputs are written; all proj
matmuls run after attention, hiding the last collective under ~38us of PE
work. Proj bias is folded into the accumulator chain (init acc = pps + bias,
final add writes bf16 directly) to kill the serialized vector tail.
Host side only shards/gathers (transpose/concat/slice).
"""

from contextlib import ExitStack

import numpy as np
import ml_dtypes

import concourse.bass as bass
import concourse.mybir as mybir
import concourse.tile as tile
from concourse import bacc
from concourse.bass_utils import run_bass_kernel_spmd

B, N, M, C, H, HD, RD = 2, 2048, 512, 1536, 12, 128, 64
EPS = 1e-6
NHL = 3               # heads per core
S = N + M             # 2560 kv tokens
KT = S // 128         # 20 kv tiles
NQC = N // 512        # 4 q-chunks of 512 (== A2A block count)
CH = 1024             # qkv-phase token chunk (bf16 moving limit)
F32 = mybir.dt.float32
F32R = mybir.dt.float32r
AF = mybir.ActivationFunctionType
ALU = mybir.AluOpType
BF16 = mybir.dt.bfloat16
NCT = C // 128        # 12 contraction tiles


def build_nc(variant=None):
    variant = variant or {}
    startup_split = variant.get("startup_split", True)
    wleaf = variant.get("wleaf", True)
    nc = bacc.Bacc("TRN2", target_bir_lowering=False, debug=False, num_devices=8)

    xT = nc.dram_tensor("xT", [C, N], BF16, kind="ExternalInput").ap()
    yT = nc.dram_tensor("yT", [C, M], BF16, kind="ExternalInput").ap()
    wqkv = nc.dram_tensor("wqkv", [C, 3 * NHL * HD], BF16, kind="ExternalInput").ap()
    wkv = nc.dram_tensor("wkv", [C, 2 * NHL * HD], BF16, kind="ExternalInput").ap()
    wproj = nc.dram_tensor("wproj", [C, C], BF16, kind="ExternalInput").ap()
    wq = nc.dram_tensor("wq", [1, HD], F32, kind="ExternalInput").ap()
    wk = nc.dram_tensor("wk", [1, HD], F32, kind="ExternalInput").ap()
    cs = nc.dram_tensor("cs", [RD, N], BF16, kind="ExternalInput").ap()
    sn = nc.dram_tensor("sn", [RD, N], BF16, kind="ExternalInput").ap()
    ywT = nc.dram_tensor("ywT", [128, M // 128], F32, kind="ExternalInput").ap()
    bpr = nc.dram_tensor("bpr", [1, C], F32, kind="ExternalInput").ap()
    onesb = nc.dram_tensor("onesb", [128, 1], BF16, kind="ExternalInput").ap()
    ones128 = nc.dram_tensor("ones128", [1, 128], F32R, kind="ExternalInput").ap()
    lywd = nc.dram_tensor("lywd", [128, M // 128], F32, kind="ExternalInput").ap()
    m0d = nc.dram_tensor("m0d", [128, 1], F32, kind="ExternalInput").ap()
    m1d = nc.dram_tensor("m1d", [128, 1], F32, kind="ExternalInput").ap()
    out = nc.dram_tensor("out", [512, C], BF16, kind="ExternalOutput").ap()

    with tile.TileContext(nc) as tc, ExitStack() as ctx:
        # ---------- outer (whole-kernel) pools ----------
        pers = ctx.enter_context(tc.tile_pool(name="persist", bufs=1))
        dram = ctx.enter_context(tc.tile_pool(name="dram", bufs=1, space="DRAM"))

        onesb_sb = pers.tile([128, 1], BF16, tag="onesb")
        nc.gpsimd.dma_start(onesb_sb[:], onesb)
        ones128_sb = pers.tile([1, 128], F32R, tag="ones128")
        nc.gpsimd.dma_start(ones128_sb[:], ones128)
        eps_sb = pers.tile([1, 1], F32, tag="eps")
        nc.vector.memset(eps_sb[:], EPS)
        wq_sb = pers.tile([128, 1], F32, tag="wq")
        nc.gpsimd.dma_start(wq_sb[:], wq.rearrange("o p -> p o"))
        wk_sb = pers.tile([128, 1], F32, tag="wk")
        nc.gpsimd.dma_start(wk_sb[:], wk.rearrange("o p -> p o"))
        m0_sb = pers.tile([128, 1], F32, tag="m0")
        nc.gpsimd.dma_start(m0_sb[:], m0d)
        m1_sb = pers.tile([128, 1], F32, tag="m1")
        nc.gpsimd.dma_start(m1_sb[:], m1d)

        # y token weights, one column per y kv tile; clamped on host
        ywT_sb = pers.tile([128, M // 128], F32, tag="ywT")
        nc.gpsimd.dma_start(ywT_sb[:], ywT)
        if not wleaf:
            # bias columns per y kv tile: log(clip(w)) computed on host
            lyw_sb = pers.tile([128, M // 128], F32, tag="lyw")
            nc.gpsimd.dma_start(lyw_sb[:], lywd)

        # persistent activations
        qn = [pers.tile([128, N], BF16, tag=f"qn{t}", name=f"qn{t}") for t in range(NHL)]
        kn = [pers.tile([128, S], BF16, tag=f"kn{t}", name=f"kn{t}") for t in range(NHL)]
        v_sb = pers.tile([128, KT * NHL * HD], BF16, tag="v")  # [kv_tile, head, hd]

        outp = ctx.enter_context(tc.tile_pool(name="osb", bufs=3))

        # ---------- phase A/B: qkv + kv, norm, rope ----------
        with ExitStack() as ab:
            csn = ab.enter_context(tc.tile_pool(name="csn", bufs=1))
            wbig = ab.enter_context(tc.tile_pool(name="wbig", bufs=1))
            xtp = ab.enter_context(tc.tile_pool(name="xt", bufs=2))
            sqp = ab.enter_context(tc.tile_pool(name="sq", bufs=2))
            smallp = ab.enter_context(tc.tile_pool(name="small", bufs=3))
            brp = ab.enter_context(tc.tile_pool(name="bcast", bufs=2))
            ropep = ab.enter_context(tc.tile_pool(name="rope", bufs=2))
            psA = ab.enter_context(tc.tile_pool(name="psA", bufs=2, space="PSUM"))
            psV = ab.enter_context(tc.tile_pool(name="psV", bufs=2, space="PSUM"))
            psS = ab.enter_context(tc.tile_pool(name="psS", bufs=1, space="PSUM"))

            def norm_head(raw_ps, dst, w_sb, rope_q0, CHc):
                """RMSNorm over partition dim (HD) + optional RoPE; [128,CHc]."""
                sq = sqp.tile([128, CH], BF16, tag="sq", name="sq")[:, :CHc]
                nc.scalar.activation(sq, raw_ps[:], AF.Square)
                ssq = psS.tile([1, CH], F32, tag="ssq", name="ssq")[:, :CHc]
                for h0 in range(0, CHc, 512):
                    hw = min(512, CHc - h0)
                    nc.tensor.matmul(
                        ssq[:, h0 : h0 + hw],
                        onesb_sb[:],
                        sq[:, h0 : h0 + hw],
                        start=True,
                        stop=True,
                    )
                inv = smallp.tile([1, CH], F32, tag="inv", name="inv")[:, :CHc]
                nc.scalar.activation(
                    inv, ssq, AF.Abs_reciprocal_sqrt, bias=eps_sb[:],
                    scale=1.0 / HD,
                )
                binv = brp.tile([128, CH], F32, tag="binv", name="binv")[:, :CHc]
                nc.gpsimd.partition_broadcast(binv, inv)
                nc.vector.scalar_tensor_tensor(
                    dst, raw_ps[:], w_sb[:], binv, op0=ALU.mult, op1=ALU.mult
                )
                if rope_q0 is not None:
                    hf = RD // 2
                    csc = cs_sb[:, rope_q0 : rope_q0 + CHc]
                    snc = sn_sb[:, rope_q0 : rope_q0 + CHc]
                    sw = ropep.tile([RD, CH], BF16, tag="sw", name="sw")[:, :CHc]
                    nc.scalar.copy(sw[0:hf, :], dst[hf:RD, :])
                    nc.scalar.copy(sw[hf:RD, :], dst[0:hf, :])
                    ma = ropep.tile([RD, CH], BF16, tag="ma", name="ma")[:, :CHc]
                    mb = ropep.tile([RD, CH], BF16, tag="mb", name="mb")[:, :CHc]
                    nc.vector.tensor_mul(ma, dst[0:RD, :], csc)
                    nc.vector.tensor_mul(mb, sw, snc)
                    nc.vector.tensor_add(dst[0:RD, :], ma, mb)

            def v_chunk(src_sb, w_sb, nqh, vt0, CHc, vscale):
                """v heads for one chunk; vscale: per-kv-tile weight cols or None."""
                voff = (nqh + NHL) * HD
                for ts in range(CHc // 128):
                    ps = psV.tile([128, NHL * HD], F32, tag="vps")
                    for ct in range(NCT):
                        nc.tensor.matmul(
                            ps[:],
                            src_sb[:, ct, ts * 128 : (ts + 1) * 128],
                            w_sb[:, ct, voff : voff + NHL * HD],
                            start=(ct == 0),
                            stop=(ct == NCT - 1),
                        )
                    kvt = vt0 + ts
                    dst = v_sb[:, kvt * NHL * HD : (kvt + 1) * NHL * HD]
                    if vscale is not None:
                        nc.vector.tensor_scalar_mul(dst, ps[:], vscale[:, ts : ts + 1])
                    else:
                        nc.vector.tensor_copy(dst, ps[:])

            def qkv_chunk(src_sb, w_sb, nqh, q0, kdst_off, vt0, rope, CHc,
                          vscale=None, vfirst=False):
                """One CHc-token chunk: q (nqh heads), k (NHL heads), v (NHL heads)."""
                if vfirst:
                    v_chunk(src_sb, w_sb, nqh, vt0, CHc, vscale)
                for t in range(nqh):
                    ps = psA.tile([128, CH], F32, tag="qk", name="qk")[:, :CHc]
                    for ct in range(NCT):
                        for h0 in range(0, CHc, 512):
                            hw = min(512, CHc - h0)
                            nc.tensor.matmul(
                                ps[:, h0 : h0 + hw],
                                w_sb[:, ct, t * HD : (t + 1) * HD],
                                src_sb[:, ct, h0 : h0 + hw],
                                start=(ct == 0),
                                stop=(ct == NCT - 1),
                            )
                    norm_head(
                        ps, qn[t][:, q0 : q0 + CHc], wq_sb,
                        q0 if rope else None, CHc,
                    )
                koff = nqh * HD
                for t in range(NHL):
                    ps = psA.tile([128, CH], F32, tag="qk", name="qk")[:, :CHc]
                    for ct in range(NCT):
                        for h0 in range(0, CHc, 512):
                            hw = min(512, CHc - h0)
                            nc.tensor.matmul(
                                ps[:, h0 : h0 + hw],
                                w_sb[:, ct, koff + t * HD : koff + (t + 1) * HD],
                                src_sb[:, ct, h0 : h0 + hw],
                                start=(ct == 0),
                                stop=(ct == NCT - 1),
                            )
                    norm_head(
                        ps,
                        kn[t][:, kdst_off : kdst_off + CHc],
                        wk_sb,
                        q0 if rope else None,
                        CHc,
                    )
                if not vfirst:
                    v_chunk(src_sb, w_sb, nqh, vt0, CHc, vscale)

            xt_first = xtp.tile([128, NCT, CH], BF16, tag="xt", name="xt_first")
            wqkv_sb = wbig.tile([128, NCT, 3 * NHL * HD], BF16, tag="wbig")
            # wkv has its own buffer and loads early (vector queue) so the
            # y phase isn't gated on a late weight fetch
            wkv_sb = wbig.tile([128, NCT, 2 * NHL * HD], BF16, tag="wkv")
            if startup_split:
                # tiny first transfers so the first matmul starts ASAP;
                # wqkv slices alternate sync/vector so two DMA queues feed
                # the startup-critical weights in parallel
                nc.sync.dma_start(wqkv_sb[:, 0, 0:HD], wqkv[0:128, 0:HD])
                nc.scalar.dma_start(xt_first[:, 0, 0:512], xT[0:128, 0:512])
                nc.sync.dma_start(
                    wqkv_sb[:, 0, HD : 3 * NHL * HD],
                    wqkv[0:128, HD : 3 * NHL * HD],
                )
                nc.scalar.dma_start(xt_first[:, 0, 512:CH], xT[0:128, 512:CH])
                for ct in range(1, NCT):
                    nc.sync.dma_start(
                        wqkv_sb[:, ct, :], wqkv[ct * 128 : (ct + 1) * 128, :]
                    )
                    nc.scalar.dma_start(
                        xt_first[:, ct, :],
                        xT[ct * 128 : (ct + 1) * 128, 0:CH],
                    )
            else:
                for ct in range(NCT):
                    nc.sync.dma_start(
                        wqkv_sb[:, ct, :], wqkv[ct * 128 : (ct + 1) * 128, :]
                    )
                    nc.scalar.dma_start(
                        xt_first[:, ct, :],
                        xT[ct * 128 : (ct + 1) * 128, 0:CH],
                    )
            cs_sb = csn.tile([RD, N], BF16, tag="cs")
            nc.gpsimd.dma_start(cs_sb[:], cs)
            sn_sb = csn.tile([RD, N], BF16, tag="sn")
            nc.gpsimd.dma_start(sn_sb[:], sn)
            # wkv on the scalar queue right behind the xt chunk-0 slices:
            # lands well before the y phase needs it
            for ct in range(NCT):
                nc.scalar.dma_start(
                    wkv_sb[:, ct, :], wkv[ct * 128 : (ct + 1) * 128, :]
                )
            # y tokens get their own right-sized buffer, fetched early (no
            # WAR on the xt pool)
            yt_sb = xtp.tile([128, NCT, M], BF16, tag="yt", bufs=1)
            for qc in range(N // CH):
                q0 = qc * CH
                if qc == 0:
                    xt_sb = xt_first
                else:
                    xt_sb = xtp.tile([128, NCT, CH], BF16, tag="xt", bufs=2)
                    nc.sync.dma_start(
                        xt_sb[:],
                        xT[:, q0 : q0 + CH].rearrange("(ct p) q -> p ct q", p=128),
                    )
                    nc.sync.dma_start(
                        yt_sb[:], yT.rearrange("(ct p) q -> p ct q", p=128)
                    )
                qkv_chunk(xt_sb, wqkv_sb, NHL, q0, q0, q0 // 128, rope=True, CHc=CH)
            qkv_chunk(
                yt_sb, wkv_sb, 0, 0, N, N // 128, rope=False, CHc=M,
                vscale=(ywT_sb if wleaf else None), vfirst=True,
            )

        # ---------- phase C: attention + per-head A2A, then projection ----------
        with ExitStack() as pc:
            expp = pc.enter_context(tc.tile_pool(name="exp", bufs=6))
            exsp = pc.enter_context(tc.tile_pool(name="exs", bufs=3))
            brp2 = pc.enter_context(tc.tile_pool(name="bcast2", bufs=2))
            smallc = pc.enter_context(tc.tile_pool(name="smallc", bufs=2))
            accp = pc.enter_context(tc.tile_pool(name="accp", bufs=1))
            pjp = pc.enter_context(tc.tile_pool(name="pjp", bufs=1))
            wpre = pc.enter_context(tc.tile_pool(name="wpre", bufs=2))

            bpr_sb = pjp.tile([1, C], F32, tag="bpr")
            nc.sync.dma_start(bpr_sb[:], bpr)
            bb_sb = pjp.tile([128, C], F32, tag="bb")
            nc.gpsimd.partition_broadcast(bb_sb[:], bpr_sb[:])

            a2a_ins = [
                dram.tile([2 * NQC, 128, 512], BF16, name=f"a2ai{t}") for t in range(NHL)
            ]
            a2a_outs = [
                dram.tile([2 * NQC, 128, 512], BF16, name=f"a2ao{t}") for t in range(NHL)
            ]
            acc = [
                accp.tile([128, 512], F32, tag=f"acc{i}", name=f"acc{i}")
                for i in range(12)
            ]
            pj = [None] * NHL

            def prefetch_w(t):
                wp = wpre.tile(
                    [128, 12, 512], BF16, tag="wpre", bufs=3, name=f"wpre{t}",
                )
                for i in range(NQC):
                    nc.sync.dma_start(
                        wp[:, 3 * i : 3 * (i + 1), :],
                        wproj[t * 512 + i * 128 : t * 512 + (i + 1) * 128, :],
                    )
                return wp

            with ExitStack() as aps:
                psSc = aps.enter_context(tc.tile_pool(name="psSc", bufs=2, space="PSUM"))
                psAv = aps.enter_context(tc.tile_pool(name="psAv", bufs=2, space="PSUM"))
                psDen = aps.enter_context(tc.tile_pool(name="psDen", bufs=1, space="PSUM"))

                def attention_head(t):
                    for qc in range(NQC):
                        av = psAv.tile([128, 512], F32, tag="av")
                        den = psDen.tile([1, 512], F32, tag="den")
                        pair_exs = []
                        quad_exs = []
                        for kp in range(KT // 2):
                            sc = psSc.tile([128, 1024], F32, tag="sc")
                            for kh in range(2):
                                kt = 2 * kp + kh
                                nc.tensor.matmul(
                                    sc[:, kh * 512 : (kh + 1) * 512],
                                    kn[t][:, kt * 128 : (kt + 1) * 128],
                                    qn[t][:, qc * 512 : (qc + 1) * 512],
                                    start=True,
                                    stop=True,
                                )
                            ex = expp.tile([128, 1024], BF16, tag="ex", bufs=9)
                            if wleaf or kp < 8:
                                nc.scalar.activation(ex[:], sc[:], AF.Exp)
                            else:
                                for kh in range(2):
                                    kt = 2 * kp + kh
                                    nc.scalar.activation(
                                        ex[:, kh * 512 : (kh + 1) * 512],
                                        sc[:, kh * 512 : (kh + 1) * 512],
                                        AF.Exp,
                                        bias=lyw_sb[:, kt - 16 : kt - 15],
                                    )
                            for kh in range(2):
                                kt = 2 * kp + kh
                                nc.tensor.matmul(
                                    av[:],
                                    v_sb[
                                        :,
                                        kt * NHL * HD
                                        + t * HD : kt * NHL * HD
                                        + (t + 1) * HD,
                                    ],
                                    ex[:, kh * 512 : (kh + 1) * 512],
                                    start=(kt == 0),
                                    stop=(kt == KT - 1),
                                )
                            exs = exsp.tile([128, 512], BF16, tag="exs", bufs=4)
                            if wleaf and kp >= 8:
                                # w-weighted leaf: exs = ex_a*w_a + ex_b*w_b
                                ca = 2 * (kp - 8)
                                tmp = exsp.tile([128, 512], BF16, tag="ytmp", bufs=2)
                                nc.vector.tensor_scalar_mul(
                                    tmp[:], ex[:, 512:1024],
                                    ywT_sb[:, ca + 1 : ca + 2],
                                )
                                nc.vector.scalar_tensor_tensor(
                                    exs[:], ex[:, 0:512],
                                    ywT_sb[:, ca : ca + 1], tmp[:],
                                    op0=ALU.mult, op1=ALU.add,
                                )
                            else:
                                nc.vector.tensor_add(
                                    exs[:], ex[:, 0:512], ex[:, 512:1024]
                                )
                            pair_exs.append(exs)
                            if len(pair_exs) == 2:
                                exq = exsp.tile([128, 512], BF16, tag="exq", bufs=5)
                                nc.vector.tensor_add(
                                    exq[:], pair_exs[0][:], pair_exs[1][:]
                                )
                                pair_exs.clear()
                                quad_exs.append(exq)
                        # reduce the 5 quads on DVE, then a single den matmul
                        while len(quad_exs) > 1:
                            nxt = []
                            for a, b in zip(quad_exs[0::2], quad_exs[1::2]):
                                s = exsp.tile([128, 512], BF16, tag="exo", bufs=3)
                                nc.vector.tensor_add(s[:], a[:], b[:])
                                nxt.append(s)
                            if len(quad_exs) % 2:
                                nxt.append(quad_exs[-1])
                            quad_exs = nxt
                        nc.tensor.matmul(
                            den[:], onesb_sb[:], quad_exs[0][:], start=True, stop=True
                        )
                        den_sb = smallc.tile([1, 512], F32R, tag="den_sb", bufs=2)
                        nc.vector.tensor_copy(den_sb[:], den[:])
                        # broadcast den across partitions via K=1 matmul
                        bden = psDen.tile([128, 512], F32, tag="den")
                        nc.tensor.matmul(
                            bden[:], ones128_sb[:], den_sb[:], start=True, stop=True
                        )
                        binv = brp2.tile([128, 512], F32, tag="binv")
                        nc.vector.reciprocal_approx_fast(binv[:], bden[:])
                        # o1/o2: per-core batch masks m0/m1 zero the wrong-batch copy
                        o1 = outp.tile([128, 512], BF16, tag="o", bufs=4)
                        nc.vector.scalar_tensor_tensor(
                            o1[:], av[:], m0_sb[:], binv[:],
                            op0=ALU.mult, op1=ALU.mult,
                        )
                        o2 = outp.tile([128, 512], BF16, tag="o", bufs=4)
                        nc.vector.scalar_tensor_tensor(
                            o2[:], av[:], m1_sb[:], binv[:],
                            op0=ALU.mult, op1=ALU.mult,
                        )
                        nc.sync.dma_start(a2a_ins[t][qc], o1[:])
                        nc.sync.dma_start(a2a_ins[t][NQC + qc], o2[:])

                folds = [None] * NHL

                def a2a_head(t):
                    nc.gpsimd.collective_compute(
                        "AllToAll",
                        ALU.bypass,
                        replica_groups=[[0, 1, 2, 3, 4, 5, 6, 7]],
                        ins=[a2a_ins[t].opt()],
                        outs=[a2a_outs[t].opt()],
                    )
                    # gather + batch-half fold: the wait-for-collective goes
                    # on queues that can't stall attention's vector/sync work.
                    # Halves (blocks i,i+4 pairs) so the fold can start as
                    # soon as its half of the gather lands.
                    pj_t = pjp.tile(
                        [128, 2 * NQC, 512], BF16, tag="pj", bufs=2, name=f"pj{t}"
                    )
                    hw = NQC // 2
                    last = t == NHL - 1
                    pjf_t = pjp.tile(
                        [128, NQC, 512], BF16, tag=f"pjf{t}", name=f"pjf{t}"
                    )
                    if last:
                        # tail-critical: gather halves ride scalar+gpsimd in
                        # parallel; folds deferred to the idle vector queue
                        # (emitted after proj1's acc adds)
                        for h0 in (0, hw):
                            eng = nc.scalar if h0 == 0 else nc.gpsimd
                            eng.dma_start(
                                pj_t[:, h0 : h0 + hw, :],
                                a2a_outs[t][h0 : h0 + hw].rearrange(
                                    "i p q -> p i q"
                                ),
                            )
                            eng.dma_start(
                                pj_t[:, NQC + h0 : NQC + h0 + hw, :],
                                a2a_outs[t][NQC + h0 : NQC + h0 + hw].rearrange(
                                    "i p q -> p i q"
                                ),
                            )

                        def fold(eng):
                            for h0 in (0, hw):
                                eng.tensor_add(
                                    pjf_t[:, h0 : h0 + hw, :],
                                    pj_t[:, h0 : h0 + hw, :],
                                    pj_t[:, NQC + h0 : NQC + h0 + hw, :],
                                )

                        folds[t] = fold
                    else:
                        nc.gpsimd.dma_start(
                            pj_t[:], a2a_outs[t].rearrange("i p q -> p i q")
                        )
                        nc.gpsimd.tensor_add(
                            pjf_t[:], pj_t[:, 0:NQC, :], pj_t[:, NQC : 2 * NQC, :]
                        )
                    pj[t] = pjf_t

                wp0 = prefetch_w(0)
                attention_head(0)
                a2a_head(0)
                wp1 = prefetch_w(1)
                attention_head(1)
                a2a_head(1)
                wp2 = prefetch_w(2)
                attention_head(2)
                a2a_head(2)

            wps = [wp0, wp1, wp2]
            with ExitStack() as pps_ctx:
                psP = pps_ctx.enter_context(
                    tc.tile_pool(name="psP", bufs=2, space="PSUM")
                )

                def proj_partial(t):
                    wp = wps[t]
                    pjf = pj[t]
                    for fc in range(3):
                        for th in range(2):
                            pps = [
                                psP.tile(
                                    [128, 512], F32, tag=f"pp{tp_}",
                                    name=f"pp{t}_{fc}_{th}_{tp_}",
                                )
                                for tp_ in range(2)
                            ]
                            for i in range(NQC):
                                for tp_ in range(2):
                                    tcc = th * 2 + tp_
                                    nc.tensor.matmul(
                                        pps[tp_][:],
                                        pjf[:, i, tcc * 128 : (tcc + 1) * 128],
                                        wp[:, 3 * i + fc, :],
                                        start=(i == 0),
                                        stop=(i == NQC - 1),
                                    )
                            for tp_ in range(2):
                                tcc = th * 2 + tp_
                                a = acc[fc * 4 + tcc]
                                if t == 0:
                                    # fold proj bias into the init add
                                    nc.vector.tensor_tensor(
                                        a[:], pps[tp_][:],
                                        bb_sb[:, fc * 512 : (fc + 1) * 512],
                                        ALU.add,
                                    )
                                elif t == 1:
                                    nc.vector.tensor_add(a[:], a[:], pps[tp_][:])
                                else:
                                    ob = outp.tile([128, 512], BF16, tag="ob")
                                    nc.vector.tensor_add(ob[:], a[:], pps[tp_][:])
                                    nc.sync.dma_start(
                                        out[
                                            tcc * 128 : (tcc + 1) * 128,
                                            fc * 512 : (fc + 1) * 512,
                                        ],
                                        ob[:],
                                    )

                proj_partial(0)
                proj_partial(1)
                # last head's batch-half fold on the now-idle vector queue,
                # after proj0/proj1's acc adds so it can't stall them
                folds[2](nc.vector)
                proj_partial(2)
    nc.compile()
    return nc


_NC_CACHE = {}


def _get_nc(variant=None):
    key = str(sorted((variant or {}).items()))
    if key not in _NC_CACHE:
        _NC_CACHE[key] = build_nc(variant)
    return _NC_CACHE[key]


def make_in_maps(x, y, pos, y_token_weights, Wqkv, Wkv, q_norm_w, k_norm_w, Wproj, bproj):
    f = np.float32
    c32 = pos[:, :, 0].T
    s32 = pos[:, :, 1].T
    csT = np.ascontiguousarray(
        np.concatenate([c32, c32], 0).astype(ml_dtypes.bfloat16))   # [64, N]
    snT = np.ascontiguousarray(
        np.concatenate([-s32, s32], 0).astype(ml_dtypes.bfloat16))  # [64, N]
    wqs = (np.asarray(q_norm_w, dtype=f) * np.float32(HD) ** -0.5).reshape(1, HD)
    wkk = np.asarray(k_norm_w, dtype=f).reshape(1, HD)
    Wp = np.asarray(Wproj, dtype=f)
    # head-permuted Wproj: row block (t, j) = rows of head 3*j+t (same all cores)
    W = np.zeros((NHL, NQC, 128, C), dtype=f)
    for t in range(NHL):
        for j in range(NQC):
            h = 3 * j + t
            W[t, j] = Wp[h * 128 : (h + 1) * 128, :]
    wproj_perm = np.ascontiguousarray(
        W.reshape(NHL * NQC * 128, C).astype(ml_dtypes.bfloat16)
    )
    in_maps = []
    for c in range(8):
        b, g = c // 4, c % 4
        heads = [3 * g + i for i in range(NHL)]
        qcols = [Wqkv[:, h * HD : (h + 1) * HD] for h in heads]
        kcols = [Wqkv[:, C + h * HD : C + (h + 1) * HD] for h in heads]
        vcols = [Wqkv[:, 2 * C + h * HD : 2 * C + (h + 1) * HD] for h in heads]
        wqkv_c = np.ascontiguousarray(
            np.concatenate(qcols + kcols + vcols, axis=1), dtype=f
        )
        kcols2 = [Wkv[:, h * HD : (h + 1) * HD] for h in heads]
        vcols2 = [Wkv[:, C + h * HD : C + (h + 1) * HD] for h in heads]
        wkv_c = np.ascontiguousarray(np.concatenate(kcols2 + vcols2, axis=1), dtype=f)
        yw = np.clip(np.asarray(y_token_weights, dtype=f)[b], 1e-4, None)
        ywc = np.ascontiguousarray(yw.reshape(M // 128, 128).T, dtype=f)
        in_maps.append(
            {
                "xT": np.ascontiguousarray(np.asarray(x)[b].T.astype(ml_dtypes.bfloat16)),
                "yT": np.ascontiguousarray(np.asarray(y)[b].T.astype(ml_dtypes.bfloat16)),
                "wqkv": wqkv_c.astype(ml_dtypes.bfloat16),
                "wkv": wkv_c.astype(ml_dtypes.bfloat16),
                "wproj": wproj_perm,
                "wq": np.ascontiguousarray(wqs),
                "wk": np.ascontiguousarray(wkk),
                "cs": csT,
                "sn": snT,
                "ywT": ywc,
                "lywd": np.ascontiguousarray(np.log(ywc)),
                "bpr": np.asarray(bproj, dtype=f).reshape(1, C),
                "onesb": np.ones((128, 1), dtype=ml_dtypes.bfloat16),
                "ones128": np.ones((1, 128), dtype=f),
                "m0d": np.full((128, 1), 1.0 if b == 0 else 0.0, dtype=f),
                "m1d": np.full((128, 1), 0.0 if b == 0 else 1.0, dtype=f),
            }
        )
    return in_maps


def kernel(x, y, pos, y_token_weights, Wqkv, Wkv, q_norm_w, k_norm_w, Wproj, bproj,
           _trace=False, _variant=None):
    x = np.asarray(x, dtype=np.float32)
    y = np.asarray(y, dtype=np.float32)
    pos = np.asarray(pos, dtype=np.float32)
    y_token_weights = np.asarray(y_token_weights, dtype=np.float32)
    nc = _get_nc(_variant)
    in_maps = make_in_maps(
        x, y, pos, y_token_weights,
        np.asarray(Wqkv), np.asarray(Wkv), np.asarray(q_norm_w),
        np.asarray(k_norm_w), np.asarray(Wproj), np.asarray(bproj),
    )
    res = run_bass_kernel_spmd(nc, in_maps, core_ids=list(range(8)), trace=_trace)
    outp = np.zeros((B, N, C), dtype=np.float32)
    for c in range(8):
        b, g = c // 4, c % 4
        outp[b, g * 512 : (g + 1) * 512, :] = np.asarray(
            res.results[c]["out"], dtype=np.float32
        )
    if _trace:
        return outp, res
    return outp


# revision 29
# speedup vs baseline: 1.0376x; 1.0319x over previous
"""Distributed Bass kernel for nn_Attention_12953621365048 (8 TRN2 NeuronCores).

Sharding: 2 batch-groups x 4 head-groups (3 heads/core).
  core c: batch b = c//4, heads 3*(c%4) .. 3*(c%4)+2
Per core: qkv/kv matmuls (transposed [dim, tok] layout), RMSNorm + RoPE,
attention with no-max softmax (scores bounded: q,k RMSNorm'd), then one
8-way AllToAll per head (wrong-batch duplicate blocks zeroed via per-core
m0/m1 sender masks) to turn head-sharding into token-sharding; receiver
folds the two batch halves (gpsimd) and runs a 12-tile projection against
head-permuted Wproj.
y-token bias folding: instead of adding log(w) to scores, v rows and the
softmax-denominator tree leaves are scaled by w (identical math, bias-free
1024-wide exps on the scalar engine).
Softmax denominator: single den matmul per (head,chunk) off a DVE add-tree,
broadcast via a K=1 matmul + fast reciprocal.
Queue discipline: o1/o2 A2A-feed DMAs on sync; a2a triggers, pj gather DMAs
(which wait on collectives) and the batch-half folds all on gpsimd so a slow
A2A can't stall attention's vector/sync work.
Each per-head A2A fires as soon as its head's outputs are written; all proj
matmuls run after attention, hiding the last collective under ~38us of PE
work. Proj bias is folded into the accumulator chain (init acc = pps + bias,
final add writes bf16 directly) to kill the serialized vector tail.
Host side only shards/gathers (transpose/concat/slice).
"""

from contextlib import ExitStack

import numpy as np
import ml_dtypes

import concourse.bass as bass
import concourse.mybir as mybir
import concourse.tile as tile
from concourse import bacc
from concourse.bass_utils import run_bass_kernel_spmd

B, N, M, C, H, HD, RD = 2, 2048, 512, 1536, 12, 128, 64
EPS = 1e-6
NHL = 3               # heads per core
S = N + M             # 2560 kv tokens
KT = S // 128         # 20 kv tiles
NQC = N // 512        # 4 q-chunks of 512 (== A2A block count)
CH = 1024             # qkv-phase token chunk (bf16 moving limit)
F32 = mybir.dt.float32
F32R = mybir.dt.float32r
AF = mybir.ActivationFunctionType
ALU = mybir.AluOpType
BF16 = mybir.dt.bfloat16
NCT = C // 128        # 12 contraction tiles


def build_nc(variant=None):
    variant = variant or {}
    startup_split = variant.get("startup_split", True)
    wleaf = variant.get("wleaf", True)
    nc = bacc.Bacc("TRN2", target_bir_lowering=False, debug=False, num_devices=8)

    xT = nc.dram_tensor("xT", [C, N], BF16, kind="ExternalInput").ap()
    yT = nc.dram_tensor("yT", [C, M], BF16, kind="ExternalInput").ap()
    wqkv = nc.dram_tensor("wqkv", [C, 3 * NHL * HD], BF16, kind="ExternalInput").ap()
    wkv = nc.dram_tensor("wkv", [C, 2 * NHL * HD], BF16, kind="ExternalInput").ap()
    wproj = nc.dram_tensor("wproj", [C, C], BF16, kind="ExternalInput").ap()
    wq = nc.dram_tensor("wq", [1, HD], F32, kind="ExternalInput").ap()
    wk = nc.dram_tensor("wk", [1, HD], F32, kind="ExternalInput").ap()
    cs = nc.dram_tensor("cs", [RD, N], BF16, kind="ExternalInput").ap()
    sn = nc.dram_tensor("sn", [RD, N], BF16, kind="ExternalInput").ap()
    ywT = nc.dram_tensor("ywT", [128, M // 128], F32, kind="ExternalInput").ap()
    bpr = nc.dram_tensor("bpr", [1, C], F32, kind="ExternalInput").ap()
    onesb = nc.dram_tensor("onesb", [128, 1], BF16, kind="ExternalInput").ap()
    ones128 = nc.dram_tensor("ones128", [1, 128], F32R, kind="ExternalInput").ap()
    lywd = nc.dram_tensor("lywd", [128, M // 128], F32, kind="ExternalInput").ap()
    m0d = nc.dram_tensor("m0d", [128, 1], F32, kind="ExternalInput").ap()
    m1d = nc.dram_tensor("m1d", [128, 1], F32, kind="ExternalInput").ap()
    out = nc.dram_tensor("out", [512, C], BF16, kind="ExternalOutput").ap()

    with tile.TileContext(nc) as tc, ExitStack() as ctx:
        # ---------- outer (whole-kernel) pools ----------
        pers = ctx.enter_context(tc.tile_pool(name="persist", bufs=1))
        dram = ctx.enter_context(tc.tile_pool(name="dram", bufs=1, space="DRAM"))

        onesb_sb = pers.tile([128, 1], BF16, tag="onesb")
        nc.gpsimd.dma_start(onesb_sb[:], onesb)
        ones128_sb = pers.tile([1, 128], F32R, tag="ones128")
        nc.gpsimd.dma_start(ones128_sb[:], ones128)
        eps_sb = pers.tile([1, 1], F32, tag="eps")
        nc.vector.memset(eps_sb[:], EPS)
        wq_sb = pers.tile([128, 1], F32, tag="wq")
        nc.gpsimd.dma_start(wq_sb[:], wq.rearrange("o p -> p o"))
        wk_sb = pers.tile([128, 1], F32, tag="wk")
        nc.gpsimd.dma_start(wk_sb[:], wk.rearrange("o p -> p o"))
        m0_sb = pers.tile([128, 1], F32, tag="m0")
        nc.gpsimd.dma_start(m0_sb[:], m0d)
        m1_sb = pers.tile([128, 1], F32, tag="m1")
        nc.gpsimd.dma_start(m1_sb[:], m1d)

        # y token weights, one column per y kv tile; clamped on host
        ywT_sb = pers.tile([128, M // 128], F32, tag="ywT")
        nc.gpsimd.dma_start(ywT_sb[:], ywT)
        if not wleaf:
            # bias columns per y kv tile: log(clip(w)) computed on host
            lyw_sb = pers.tile([128, M // 128], F32, tag="lyw")
            nc.gpsimd.dma_start(lyw_sb[:], lywd)

        # persistent activations
        qn = [pers.tile([128, N], BF16, tag=f"qn{t}", name=f"qn{t}") for t in range(NHL)]
        kn = [pers.tile([128, S], BF16, tag=f"kn{t}", name=f"kn{t}") for t in range(NHL)]
        v_sb = pers.tile([128, KT * NHL * HD], BF16, tag="v")  # [kv_tile, head, hd]

        outp = ctx.enter_context(tc.tile_pool(name="osb", bufs=3))

        # ---------- phase A/B: qkv + kv, norm, rope ----------
        with ExitStack() as ab:
            csn = ab.enter_context(tc.tile_pool(name="csn", bufs=1))
            wbig = ab.enter_context(tc.tile_pool(name="wbig", bufs=1))
            xtp = ab.enter_context(tc.tile_pool(name="xt", bufs=2))
            sqp = ab.enter_context(tc.tile_pool(name="sq", bufs=2))
            smallp = ab.enter_context(tc.tile_pool(name="small", bufs=3))
            brp = ab.enter_context(tc.tile_pool(name="bcast", bufs=2))
            ropep = ab.enter_context(tc.tile_pool(name="rope", bufs=2))
            psA = ab.enter_context(tc.tile_pool(name="psA", bufs=2, space="PSUM"))
            psV = ab.enter_context(tc.tile_pool(name="psV", bufs=2, space="PSUM"))
            psS = ab.enter_context(tc.tile_pool(name="psS", bufs=1, space="PSUM"))

            def norm_head(raw_ps, dst, w_sb, rope_q0, CHc):
                """RMSNorm over partition dim (HD) + optional RoPE; [128,CHc]."""
                sq = sqp.tile([128, CH], BF16, tag="sq", name="sq")[:, :CHc]
                nc.scalar.activation(sq, raw_ps[:], AF.Square)
                ssq = psS.tile([1, CH], F32, tag="ssq", name="ssq")[:, :CHc]
                for h0 in range(0, CHc, 512):
                    hw = min(512, CHc - h0)
                    nc.tensor.matmul(
                        ssq[:, h0 : h0 + hw],
                        onesb_sb[:],
                        sq[:, h0 : h0 + hw],
                        start=True,
                        stop=True,
                    )
                inv = smallp.tile([1, CH], F32, tag="inv", name="inv")[:, :CHc]
                nc.scalar.activation(
                    inv, ssq, AF.Abs_reciprocal_sqrt, bias=eps_sb[:],
                    scale=1.0 / HD,
                )
                binv = brp.tile([128, CH], F32, tag="binv", name="binv")[:, :CHc]
                nc.gpsimd.partition_broadcast(binv, inv)
                nc.vector.scalar_tensor_tensor(
                    dst, raw_ps[:], w_sb[:], binv, op0=ALU.mult, op1=ALU.mult
                )
                if rope_q0 is not None:
                    hf = RD // 2
                    csc = cs_sb[:, rope_q0 : rope_q0 + CHc]
                    snc = sn_sb[:, rope_q0 : rope_q0 + CHc]
                    sw = ropep.tile([RD, CH], BF16, tag="sw", name="sw")[:, :CHc]
                    nc.scalar.copy(sw[0:hf, :], dst[hf:RD, :])
                    nc.scalar.copy(sw[hf:RD, :], dst[0:hf, :])
                    ma = ropep.tile([RD, CH], BF16, tag="ma", name="ma")[:, :CHc]
                    mb = ropep.tile([RD, CH], BF16, tag="mb", name="mb")[:, :CHc]
                    nc.vector.tensor_mul(ma, dst[0:RD, :], csc)
                    nc.vector.tensor_mul(mb, sw, snc)
                    nc.vector.tensor_add(dst[0:RD, :], ma, mb)

            def v_chunk(src_sb, w_sb, nqh, vt0, CHc, vscale):
                """v heads for one chunk; vscale: per-kv-tile weight cols or None."""
                voff = (nqh + NHL) * HD
                for ts in range(CHc // 128):
                    ps = psV.tile([128, NHL * HD], F32, tag="vps")
                    for ct in range(NCT):
                        nc.tensor.matmul(
                            ps[:],
                            src_sb[:, ct, ts * 128 : (ts + 1) * 128],
                            w_sb[:, ct, voff : voff + NHL * HD],
                            start=(ct == 0),
                            stop=(ct == NCT - 1),
                        )
                    kvt = vt0 + ts
                    dst = v_sb[:, kvt * NHL * HD : (kvt + 1) * NHL * HD]
                    if vscale is not None:
                        nc.vector.tensor_scalar_mul(dst, ps[:], vscale[:, ts : ts + 1])
                    else:
                        nc.vector.tensor_copy(dst, ps[:])

            def qkv_chunk(src_sb, w_sb, nqh, q0, kdst_off, vt0, rope, CHc,
                          vscale=None, vfirst=False):
                """One CHc-token chunk: q (nqh heads), k (NHL heads), v (NHL heads)."""
                if vfirst:
                    v_chunk(src_sb, w_sb, nqh, vt0, CHc, vscale)
                for t in range(nqh):
                    ps = psA.tile([128, CH], F32, tag="qk", name="qk")[:, :CHc]
                    for ct in range(NCT):
                        for h0 in range(0, CHc, 512):
                            hw = min(512, CHc - h0)
                            nc.tensor.matmul(
                                ps[:, h0 : h0 + hw],
                                w_sb[:, ct, t * HD : (t + 1) * HD],
                                src_sb[:, ct, h0 : h0 + hw],
                                start=(ct == 0),
                                stop=(ct == NCT - 1),
                            )
                    norm_head(
                        ps, qn[t][:, q0 : q0 + CHc], wq_sb,
                        q0 if rope else None, CHc,
                    )
                koff = nqh * HD
                for t in range(NHL):
                    ps = psA.tile([128, CH], F32, tag="qk", name="qk")[:, :CHc]
                    for ct in range(NCT):
                        for h0 in range(0, CHc, 512):
                            hw = min(512, CHc - h0)
                            nc.tensor.matmul(
                                ps[:, h0 : h0 + hw],
                                w_sb[:, ct, koff + t * HD : koff + (t + 1) * HD],
                                src_sb[:, ct, h0 : h0 + hw],
                                start=(ct == 0),
                                stop=(ct == NCT - 1),
                            )
                    norm_head(
                        ps,
                        kn[t][:, kdst_off : kdst_off + CHc],
                        wk_sb,
                        q0 if rope else None,
                        CHc,
                    )
                if not vfirst:
                    v_chunk(src_sb, w_sb, nqh, vt0, CHc, vscale)

            xt_first = xtp.tile([128, NCT, CH], BF16, tag="xt", name="xt_first")
            wqkv_sb = wbig.tile([128, NCT, 3 * NHL * HD], BF16, tag="wbig")
            # wkv has its own buffer and loads early (vector queue) so the
            # y phase isn't gated on a late weight fetch
            wkv_sb = wbig.tile([128, NCT, 2 * NHL * HD], BF16, tag="wkv")
            if startup_split:
                # feed weights in compute order: all q columns first, then
                # k+v, so the q-head matmul chains are never starved while
                # k/v columns they don't yet need hog the queue
                QW = NHL * HD
                for ct in range(NCT):
                    nc.sync.dma_start(
                        wqkv_sb[:, ct, 0:QW],
                        wqkv[ct * 128 : (ct + 1) * 128, 0:QW],
                    )
                    nc.scalar.dma_start(
                        xt_first[:, ct, :],
                        xT[ct * 128 : (ct + 1) * 128, 0:CH],
                    )
                for ct in range(NCT):
                    nc.sync.dma_start(
                        wqkv_sb[:, ct, QW : 3 * QW],
                        wqkv[ct * 128 : (ct + 1) * 128, QW : 3 * QW],
                    )
            else:
                for ct in range(NCT):
                    nc.sync.dma_start(
                        wqkv_sb[:, ct, :], wqkv[ct * 128 : (ct + 1) * 128, :]
                    )
                    nc.scalar.dma_start(
                        xt_first[:, ct, :],
                        xT[ct * 128 : (ct + 1) * 128, 0:CH],
                    )
            cs_sb = csn.tile([RD, N], BF16, tag="cs")
            nc.gpsimd.dma_start(cs_sb[:], cs)
            sn_sb = csn.tile([RD, N], BF16, tag="sn")
            nc.gpsimd.dma_start(sn_sb[:], sn)
            # y tokens get their own right-sized buffer, fetched early (no
            # WAR on the xt pool)
            yt_sb = xtp.tile([128, NCT, M], BF16, tag="yt", bufs=1)
            for qc in range(N // CH):
                q0 = qc * CH
                if qc == 0:
                    xt_sb = xt_first
                else:
                    xt_sb = xtp.tile([128, NCT, CH], BF16, tag="xt", bufs=2)
                    nc.sync.dma_start(
                        xt_sb[:],
                        xT[:, q0 : q0 + CH].rearrange("(ct p) q -> p ct q", p=128),
                    )
                    nc.sync.dma_start(
                        yt_sb[:], yT.rearrange("(ct p) q -> p ct q", p=128)
                    )
                qkv_chunk(xt_sb, wqkv_sb, NHL, q0, q0, q0 // 128, rope=True, CHc=CH)
                if qc == 0:
                    # wkv after chunk 0's emission: doesn't compete with the
                    # startup-critical transfers, lands well before the y phase
                    for ct in range(NCT):
                        nc.scalar.dma_start(
                            wkv_sb[:, ct, :], wkv[ct * 128 : (ct + 1) * 128, :]
                        )
            qkv_chunk(
                yt_sb, wkv_sb, 0, 0, N, N // 128, rope=False, CHc=M,
                vscale=(ywT_sb if wleaf else None), vfirst=True,
            )

        # ---------- phase C: attention + per-head A2A, then projection ----------
        with ExitStack() as pc:
            expp = pc.enter_context(tc.tile_pool(name="exp", bufs=6))
            exsp = pc.enter_context(tc.tile_pool(name="exs", bufs=3))
            brp2 = pc.enter_context(tc.tile_pool(name="bcast2", bufs=2))
            smallc = pc.enter_context(tc.tile_pool(name="smallc", bufs=2))
            accp = pc.enter_context(tc.tile_pool(name="accp", bufs=1))
            pjp = pc.enter_context(tc.tile_pool(name="pjp", bufs=1))
            wpre = pc.enter_context(tc.tile_pool(name="wpre", bufs=2))

            bpr_sb = pjp.tile([1, C], F32, tag="bpr")
            nc.sync.dma_start(bpr_sb[:], bpr)
            bb_sb = pjp.tile([128, C], F32, tag="bb")
            nc.gpsimd.partition_broadcast(bb_sb[:], bpr_sb[:])

            a2a_ins = [
                dram.tile([2 * NQC, 128, 512], BF16, name=f"a2ai{t}") for t in range(NHL)
            ]
            a2a_outs = [
                dram.tile([2 * NQC, 128, 512], BF16, name=f"a2ao{t}") for t in range(NHL)
            ]
            acc = [
                accp.tile([128, 512], F32, tag=f"acc{i}", name=f"acc{i}")
                for i in range(12)
            ]
            pj = [None] * NHL

            def prefetch_w(t):
                wp = wpre.tile(
                    [128, 12, 512], BF16, tag="wpre", bufs=3, name=f"wpre{t}",
                )
                for i in range(NQC):
                    nc.sync.dma_start(
                        wp[:, 3 * i : 3 * (i + 1), :],
                        wproj[t * 512 + i * 128 : t * 512 + (i + 1) * 128, :],
                    )
                return wp

            with ExitStack() as aps:
                psSc = aps.enter_context(tc.tile_pool(name="psSc", bufs=2, space="PSUM"))
                psAv = aps.enter_context(tc.tile_pool(name="psAv", bufs=2, space="PSUM"))
                psDen = aps.enter_context(tc.tile_pool(name="psDen", bufs=1, space="PSUM"))

                pend = [None]

                def attention_head(t):
                    for qc in range(NQC):
                        av = psAv.tile([128, 512], F32, tag="av")
                        pair_exs = []
                        quad_exs = []
                        for kp in range(KT // 2):
                            sc = psSc.tile([128, 1024], F32, tag="sc")
                            for kh in range(2):
                                kt = 2 * kp + kh
                                nc.tensor.matmul(
                                    sc[:, kh * 512 : (kh + 1) * 512],
                                    kn[t][:, kt * 128 : (kt + 1) * 128],
                                    qn[t][:, qc * 512 : (qc + 1) * 512],
                                    start=True,
                                    stop=True,
                                )
                            if kp == 1 and pend[0] is not None:
                                # previous unit's den chain emitted after this
                                # unit's first score pairs: the den/bden
                                # matmuls no longer stall the next unit's
                                # first exp at the boundary
                                pend[0]()
                                pend[0] = None
                            ex = expp.tile([128, 1024], BF16, tag="ex", bufs=9)
                            if wleaf or kp < 8:
                                nc.scalar.activation(ex[:], sc[:], AF.Exp)
                            else:
                                for kh in range(2):
                                    kt = 2 * kp + kh
                                    nc.scalar.activation(
                                        ex[:, kh * 512 : (kh + 1) * 512],
                                        sc[:, kh * 512 : (kh + 1) * 512],
                                        AF.Exp,
                                        bias=lyw_sb[:, kt - 16 : kt - 15],
                                    )
                            for kh in range(2):
                                kt = 2 * kp + kh
                                nc.tensor.matmul(
                                    av[:],
                                    v_sb[
                                        :,
                                        kt * NHL * HD
                                        + t * HD : kt * NHL * HD
                                        + (t + 1) * HD,
                                    ],
                                    ex[:, kh * 512 : (kh + 1) * 512],
                                    start=(kt == 0),
                                    stop=(kt == KT - 1),
                                )
                            exs = exsp.tile([128, 512], BF16, tag="exs", bufs=4)
                            if wleaf and kp >= 8:
                                # w-weighted leaf: exs = ex_a*w_a + ex_b*w_b
                                ca = 2 * (kp - 8)
                                tmp = exsp.tile([128, 512], BF16, tag="ytmp", bufs=2)
                                nc.vector.tensor_scalar_mul(
                                    tmp[:], ex[:, 512:1024],
                                    ywT_sb[:, ca + 1 : ca + 2],
                                )
                                nc.vector.scalar_tensor_tensor(
                                    exs[:], ex[:, 0:512],
                                    ywT_sb[:, ca : ca + 1], tmp[:],
                                    op0=ALU.mult, op1=ALU.add,
                                )
                            else:
                                nc.vector.tensor_add(
                                    exs[:], ex[:, 0:512], ex[:, 512:1024]
                                )
                            pair_exs.append(exs)
                            if len(pair_exs) == 2:
                                exq = exsp.tile([128, 512], BF16, tag="exq", bufs=5)
                                nc.vector.tensor_add(
                                    exq[:], pair_exs[0][:], pair_exs[1][:]
                                )
                                pair_exs.clear()
                                quad_exs.append(exq)
                        # reduce the 5 quads on DVE, then a single den matmul
                        while len(quad_exs) > 1:
                            nxt = []
                            for a, b in zip(quad_exs[0::2], quad_exs[1::2]):
                                s = exsp.tile([128, 512], BF16, tag="exo", bufs=3)
                                nc.vector.tensor_add(s[:], a[:], b[:])
                                nxt.append(s)
                            if len(quad_exs) % 2:
                                nxt.append(quad_exs[-1])
                            quad_exs = nxt
                        def fin(t=t, qc=qc, av=av, qx=quad_exs[0]):
                            den = psDen.tile([1, 512], F32, tag="den")
                            nc.tensor.matmul(
                                den[:], onesb_sb[:], qx[:], start=True, stop=True
                            )
                            den_sb = smallc.tile([1, 512], F32R, tag="den_sb", bufs=2)
                            nc.vector.tensor_copy(den_sb[:], den[:])
                            # broadcast den across partitions via K=1 matmul
                            bden = psDen.tile([128, 512], F32, tag="den")
                            nc.tensor.matmul(
                                bden[:], ones128_sb[:], den_sb[:],
                                start=True, stop=True,
                            )
                            binv = brp2.tile([128, 512], F32, tag="binv")
                            nc.vector.reciprocal_approx_fast(binv[:], bden[:])
                            # o1/o2: per-core batch masks zero the wrong-batch copy
                            o1 = outp.tile([128, 512], BF16, tag="o", bufs=4)
                            nc.vector.scalar_tensor_tensor(
                                o1[:], av[:], m0_sb[:], binv[:],
                                op0=ALU.mult, op1=ALU.mult,
                            )
                            o2 = outp.tile([128, 512], BF16, tag="o", bufs=4)
                            nc.vector.scalar_tensor_tensor(
                                o2[:], av[:], m1_sb[:], binv[:],
                                op0=ALU.mult, op1=ALU.mult,
                            )
                            nc.sync.dma_start(a2a_ins[t][qc], o1[:])
                            nc.sync.dma_start(a2a_ins[t][NQC + qc], o2[:])

                        if qc == NQC - 1:
                            # last unit of the head feeds this head's A2A:
                            # finish immediately
                            fin()
                        else:
                            pend[0] = fin

                folds = [None] * NHL

                def a2a_head(t):
                    nc.gpsimd.collective_compute(
                        "AllToAll",
                        ALU.bypass,
                        replica_groups=[[0, 1, 2, 3, 4, 5, 6, 7]],
                        ins=[a2a_ins[t].opt()],
                        outs=[a2a_outs[t].opt()],
                    )
                    # gather + batch-half fold: the wait-for-collective goes
                    # on queues that can't stall attention's vector/sync work.
                    # Halves (blocks i,i+4 pairs) so the fold can start as
                    # soon as its half of the gather lands.
                    pj_t = pjp.tile(
                        [128, 2 * NQC, 512], BF16, tag="pj", bufs=3, name=f"pj{t}"
                    )
                    hw = NQC // 2
                    last = t == NHL - 1
                    pjf_t = pjp.tile(
                        [128, NQC, 512], BF16, tag=f"pjf{t}", name=f"pjf{t}"
                    )
                    if last:
                        # tail-critical: gather halves ride scalar+gpsimd in
                        # parallel
                        for h0 in (0, hw):
                            eng = nc.scalar if h0 == 0 else nc.gpsimd
                            eng.dma_start(
                                pj_t[:, h0 : h0 + hw, :],
                                a2a_outs[t][h0 : h0 + hw].rearrange(
                                    "i p q -> p i q"
                                ),
                            )
                            eng.dma_start(
                                pj_t[:, NQC + h0 : NQC + h0 + hw, :],
                                a2a_outs[t][NQC + h0 : NQC + h0 + hw].rearrange(
                                    "i p q -> p i q"
                                ),
                            )
                    else:
                        nc.gpsimd.dma_start(
                            pj_t[:], a2a_outs[t].rearrange("i p q -> p i q")
                        )

                    # all folds on vector, deferred past attention so their
                    # wait-for-gather never stalls mid-attention vector work
                    # (a gpsimd fold measurably slowed concurrent DVE ops)
                    def fold(eng, pjf_t=pjf_t, pj_t=pj_t):
                        for h0 in (0, hw):
                            eng.tensor_add(
                                pjf_t[:, h0 : h0 + hw, :],
                                pj_t[:, h0 : h0 + hw, :],
                                pj_t[:, NQC + h0 : NQC + h0 + hw, :],
                            )

                    folds[t] = fold
                    pj[t] = pjf_t

                wp0 = prefetch_w(0)
                attention_head(0)
                a2a_head(0)
                wp1 = prefetch_w(1)
                attention_head(1)
                a2a_head(1)
                wp2 = prefetch_w(2)
                attention_head(2)
                a2a_head(2)

            wps = [wp0, wp1, wp2]
            with ExitStack() as pps_ctx:
                psP = pps_ctx.enter_context(
                    tc.tile_pool(name="psP", bufs=2, space="PSUM")
                )

                def proj_partial(t):
                    wp = wps[t]
                    pjf = pj[t]
                    for fc in range(3):
                        for th in range(2):
                            pps = [
                                psP.tile(
                                    [128, 512], F32, tag=f"pp{tp_}",
                                    name=f"pp{t}_{fc}_{th}_{tp_}",
                                )
                                for tp_ in range(2)
                            ]
                            for i in range(NQC):
                                for tp_ in range(2):
                                    tcc = th * 2 + tp_
                                    nc.tensor.matmul(
                                        pps[tp_][:],
                                        pjf[:, i, tcc * 128 : (tcc + 1) * 128],
                                        wp[:, 3 * i + fc, :],
                                        start=(i == 0),
                                        stop=(i == NQC - 1),
                                    )
                            for tp_ in range(2):
                                tcc = th * 2 + tp_
                                a = acc[fc * 4 + tcc]
                                if t == 0:
                                    # fold proj bias into the init add
                                    nc.vector.tensor_tensor(
                                        a[:], pps[tp_][:],
                                        bb_sb[:, fc * 512 : (fc + 1) * 512],
                                        ALU.add,
                                    )
                                elif t == 1:
                                    nc.vector.tensor_add(a[:], a[:], pps[tp_][:])
                                else:
                                    ob = outp.tile([128, 512], BF16, tag="ob")
                                    nc.vector.tensor_add(ob[:], a[:], pps[tp_][:])
                                    nc.sync.dma_start(
                                        out[
                                            tcc * 128 : (tcc + 1) * 128,
                                            fc * 512 : (fc + 1) * 512,
                                        ],
                                        ob[:],
                                    )

                # folds 0/1 first (their gathers completed during attention,
                # so the waits are already satisfied); fold 2 after proj1's
                # acc adds so its wait-for-gather can't stall them
                folds[0](nc.vector)
                folds[1](nc.vector)
                proj_partial(0)
                proj_partial(1)
                folds[2](nc.vector)
                proj_partial(2)
    nc.compile()
    return nc


_NC_CACHE = {}


def _get_nc(variant=None):
    key = str(sorted((variant or {}).items()))
    if key not in _NC_CACHE:
        _NC_CACHE[key] = build_nc(variant)
    return _NC_CACHE[key]


def make_in_maps(x, y, pos, y_token_weights, Wqkv, Wkv, q_norm_w, k_norm_w, Wproj, bproj):
    f = np.float32
    c32 = pos[:, :, 0].T
    s32 = pos[:, :, 1].T
    csT = np.ascontiguousarray(
        np.concatenate([c32, c32], 0).astype(ml_dtypes.bfloat16))   # [64, N]
    snT = np.ascontiguousarray(
        np.concatenate([-s32, s32], 0).astype(ml_dtypes.bfloat16))  # [64, N]
    wqs = (np.asarray(q_norm_w, dtype=f) * np.float32(HD) ** -0.5).reshape(1, HD)
    wkk = np.asarray(k_norm_w, dtype=f).reshape(1, HD)
    Wp = np.asarray(Wproj, dtype=f)
    # head-permuted Wproj: row block (t, j) = rows of head 3*j+t (same all cores)
    W = np.zeros((NHL, NQC, 128, C), dtype=f)
    for t in range(NHL):
        for j in range(NQC):
            h = 3 * j + t
            W[t, j] = Wp[h * 128 : (h + 1) * 128, :]
    wproj_perm = np.ascontiguousarray(
        W.reshape(NHL * NQC * 128, C).astype(ml_dtypes.bfloat16)
    )
    in_maps = []
    for c in range(8):
        b, g = c // 4, c % 4
        heads = [3 * g + i for i in range(NHL)]
        qcols = [Wqkv[:, h * HD : (h + 1) * HD] for h in heads]
        kcols = [Wqkv[:, C + h * HD : C + (h + 1) * HD] for h in heads]
        vcols = [Wqkv[:, 2 * C + h * HD : 2 * C + (h + 1) * HD] for h in heads]
        wqkv_c = np.ascontiguousarray(
            np.concatenate(qcols + kcols + vcols, axis=1), dtype=f
        )
        kcols2 = [Wkv[:, h * HD : (h + 1) * HD] for h in heads]
        vcols2 = [Wkv[:, C + h * HD : C + (h + 1) * HD] for h in heads]
        wkv_c = np.ascontiguousarray(np.concatenate(kcols2 + vcols2, axis=1), dtype=f)
        yw = np.clip(np.asarray(y_token_weights, dtype=f)[b], 1e-4, None)
        ywc = np.ascontiguousarray(yw.reshape(M // 128, 128).T, dtype=f)
        in_maps.append(
            {
                "xT": np.ascontiguousarray(np.asarray(x)[b].T.astype(ml_dtypes.bfloat16)),
                "yT": np.ascontiguousarray(np.asarray(y)[b].T.astype(ml_dtypes.bfloat16)),
                "wqkv": wqkv_c.astype(ml_dtypes.bfloat16),
                "wkv": wkv_c.astype(ml_dtypes.bfloat16),
                "wproj": wproj_perm,
                "wq": np.ascontiguousarray(wqs),
                "wk": np.ascontiguousarray(wkk),
                "cs": csT,
                "sn": snT,
                "ywT": ywc,
                "lywd": np.ascontiguousarray(np.log(ywc)),
                "bpr": np.asarray(bproj, dtype=f).reshape(1, C),
                "onesb": np.ones((128, 1), dtype=ml_dtypes.bfloat16),
                "ones128": np.ones((1, 128), dtype=f),
                "m0d": np.full((128, 1), 1.0 if b == 0 else 0.0, dtype=f),
                "m1d": np.full((128, 1), 0.0 if b == 0 else 1.0, dtype=f),
            }
        )
    return in_maps


def kernel(x, y, pos, y_token_weights, Wqkv, Wkv, q_norm_w, k_norm_w, Wproj, bproj,
           _trace=False, _variant=None):
    x = np.asarray(x, dtype=np.float32)
    y = np.asarray(y, dtype=np.float32)
    pos = np.asarray(pos, dtype=np.float32)
    y_token_weights = np.asarray(y_token_weights, dtype=np.float32)
    nc = _get_nc(_variant)
    in_maps = make_in_maps(
        x, y, pos, y_token_weights,
        np.asarray(Wqkv), np.asarray(Wkv), np.asarray(q_norm_w),
        np.asarray(k_norm_w), np.asarray(Wproj), np.asarray(bproj),
    )
    res = run_bass_kernel_spmd(nc, in_maps, core_ids=list(range(8)), trace=_trace)
    outp = np.zeros((B, N, C), dtype=np.float32)
    for c in range(8):
        b, g = c // 4, c % 4
        outp[b, g * 512 : (g + 1) * 512, :] = np.asarray(
            res.results[c]["out"], dtype=np.float32
        )
    if _trace:
        return outp, res
    return outp
